# revision 6
# baseline (speedup 1.0000x reference)
"""GNN message-passing (CompGCN edge-softmax) TRN2 kernel — V2.

Per-core structure (edges sharded by dst range, 8 cores):
- src rows: GPSIMD dma_gather fp32, rotated over the 4 SWDGE queues so the
  four Q7 core-pairs generate descriptors in parallel.
- dst rows: dual-bf16 one-hot matmuls (exact score), one-hot SHIPPED from
  host (no IS_EQ).
- rel rows: dual-bf16 run-window matmuls with tight shipped one-hot windows
  (partition-offset PSUM outputs).
- comp/score: DVE fp32 reading rel/dst PSUM directly (no PSUM->SBUF copies).
- aggregation: comp_es = comp_bf * es_full (bf16 2x); accT += comp_es^T @
  ohtT (shipped [slot,node] one-hot); den += ohtT^T @ es_col.
- out = tanh((accT^T @ W) / den).
"""

import numpy as np

N_ENT = 50000
N_REL = 1000
H = 128
P = 128
NQ = 8
N_CORES = 8
NPC = N_ENT // N_CORES
LO_ROWS = 32768
G = 8  # chunks per PSUM group

_cache = {}


def _bfsplit(x):
    import ml_dtypes
    hi = x.astype(np.float32).astype(ml_dtypes.bfloat16)
    lo = (x.astype(np.float32) - hi.astype(np.float32)).astype(ml_dtypes.bfloat16)
    return hi, lo


def _idx_to_gather_layout(arr):
    a = arr.reshape(-1, 16).T.astype(np.int16)
    return np.tile(a, (8, 1))


def _build_program(npc, n_ent, lo_rows, s_los, s_his, runs_all):
    """runs_all[b] = list of (c, q, a, z, roff): chunk c, rel chunk q,
    slot range [a, z) within block, roht col offset roff."""
    import concourse.bacc as bacc
    import concourse.mybir as mybir
    import concourse.tile as tile
    import concourse.bass as bass

    f32 = mybir.dt.float32
    bf16 = mybir.dt.bfloat16
    f8 = mybir.dt.float8e4
    i16 = mybir.dt.int16
    n_blocks = len(s_los)
    s_tot = [a + b for a, b in zip(s_los, s_his)]
    S_max = max(s_tot)
    W_chunks = sum(s_tot)
    R_cols = sum(len(runs) for runs in runs_all) * P
    R_max = max(len(runs) for runs in runs_all) * P

    nc = bacc.Bacc("TRN2", target_bir_lowering=False, debug=False,
                   num_devices=N_CORES, num_swdge_queues=4)

    ent = nc.dram_tensor("ent", [n_ent, H], f32, kind="ExternalInput")
    elh_in = nc.dram_tensor("ent_loc_hi", [npc, H], bf16, kind="ExternalInput")
    ell_in = nc.dram_tensor("ent_loc_lo", [npc, H], bf16, kind="ExternalInput")
    rlh_in = nc.dram_tensor("rel_hi", [P, NQ, H], bf16, kind="ExternalInput")
    rll_in = nc.dram_tensor("rel_lo", [P, NQ, H], bf16, kind="ExternalInput")
    w_in = nc.dram_tensor("w", [H, H], f32, kind="ExternalInput")
    sgi_in = nc.dram_tensor("src_gi", [P, W_chunks * 8], i16,
                            kind="ExternalInput")
    oht_in = nc.dram_tensor("oht", [P, W_chunks * P], f8,
                            kind="ExternalInput")
    ohtT_in = nc.dram_tensor("ohtT", [P, W_chunks, P], f8,
                             kind="ExternalInput")
    roht_in = nc.dram_tensor("roht", [P, R_cols], f8, kind="ExternalInput")
    out = nc.dram_tensor("out", [npc, H], f32, kind="ExternalOutput")

    def bc(ap, dims):
        return bass.AP(ap.tensor, ap.offset, dims)

    with tile.TileContext(nc) as tc:
        with (
            tc.tile_pool(name="const", bufs=1) as constp,
            tc.tile_pool(name="idx", bufs=1) as idxp,
            tc.tile_pool(name="data", bufs=3) as datap,
            tc.tile_pool(name="small", bufs=2) as smallp,
            tc.tile_pool(name="psg", bufs=1, space="PSUM") as psg,
            tc.tile_pool(name="psa", bufs=1, space="PSUM") as psa,
        ):
            w_t = constp.tile([H, H], f32)
            nc.sync.dma_start(w_t[:], w_in[:])
            rlh_t = constp.tile([P, NQ, H], bf16)
            nc.sync.dma_start(rlh_t[:], rlh_in[:])
            rll_t = constp.tile([P, NQ, H], bf16)
            nc.sync.dma_start(rll_t[:], rll_in[:])

            sgi_t = idxp.tile([P, W_chunks * 8], i16)
            nc.sync.dma_start(sgi_t[:], sgi_in[:])

            coff = 0
            roff0 = 0
            for b in range(n_blocks):
                base = b * P
                nodes_b = min(P, npc - base)
                s_lo, s_hi = s_los[b], s_his[b]
                S = s_lo + s_hi
                ns = S * P
                runs = runs_all[b]
                r_cols_b = len(runs) * P

                src_rows = datap.tile([P, S_max, H], f32, tag="src")
                scratch = datap.tile([P, S_max, H], f32, tag="scratch")
                comp_bf = datap.tile([P, S_max, H], bf16, tag="compbf")
                es_full = datap.tile([P, S_max, H], bf16, tag="esfull")
                comp_es = datap.tile([P, S_max, H], bf16, tag="compes")
                oht_t = datap.tile([P, S_max * P], f8, tag="OHT")
                ohtT_t = datap.tile([P, S_max, P], f8, tag="OHTT")
                roht_t = datap.tile([P, R_max], f8, tag="ROHT")
                ehi_t = datap.tile([P, H], bf16, tag="ehi")
                elo_t = datap.tile([P, H], bf16, tag="elo")
                score = smallp.tile([P, S_max], f32, tag="score")
                es = smallp.tile([P, S_max], bf16, tag="es")

                if s_lo > 0:
                    nc.gpsimd.dma_gather(
                        src_rows[:, 0:s_lo, :], ent[0:lo_rows, :],
                        sgi_t[:, coff * 8:(coff + s_lo) * 8],
                        s_lo * P, s_lo * P, H, single_packet=False,
                        queue_num=(2 * b) % 4)
                if s_hi > 0:
                    nc.gpsimd.dma_gather(
                        src_rows[:, s_lo:S, :], ent[lo_rows:n_ent, :],
                        sgi_t[:, (coff + s_lo) * 8:(coff + S) * 8],
                        s_hi * P, s_hi * P, H, single_packet=False,
                        queue_num=(2 * b + 1) % 4)
                nc.sync.dma_start(oht_t[:, 0:ns],
                                  oht_in[:, coff * P:coff * P + ns])
                nc.sync.dma_start(ohtT_t[:, 0:S, :],
                                  ohtT_in[:, coff:coff + S, :])
                nc.sync.dma_start(roht_t[:, 0:r_cols_b],
                                  roht_in[:, roff0:roff0 + r_cols_b])
                if nodes_b < P:
                    nc.vector.memset(ehi_t[:], 0.0)
                    nc.vector.memset(elo_t[:], 0.0)
                nc.sync.dma_start(ehi_t[:nodes_b, :],
                                  elh_in[base:base + nodes_b, :])
                nc.sync.dma_start(elo_t[:nodes_b, :],
                                  ell_in[base:base + nodes_b, :])

                # per-group: dst rows + rel rows (PSUM) -> comp, prod, score
                n_groups = (S + G - 1) // G
                for g in range(n_groups):
                    c0, c1 = g * G, min((g + 1) * G, S)
                    gw = c1 - c0
                    drows_ps = psg.tile([P, G, H], f32, tag="drows")
                    rel_ps = psg.tile([P, G, H], f32, tag="relps")
                    for c in range(c0, c1):
                        lhs = oht_t[:, c * P:(c + 1) * P]
                        nc.tensor.matmul(drows_ps[:, c - c0, :], lhsT=lhs,
                                         rhs=ehi_t[:], start=True, stop=False)
                        nc.tensor.matmul(drows_ps[:, c - c0, :], lhsT=lhs,
                                         rhs=elo_t[:], start=False, stop=True)
                        cruns = [r for r in runs if r[0] == c]
                        for i, (cc, q, a, z, ro) in enumerate(cruns):
                            lhsr = roht_t[:, ro:ro + P]
                            outr = rel_ps[:, c - c0, :]
                            nc.tensor.matmul(outr, lhsT=lhsr,
                                             rhs=rlh_t[:, q, :],
                                             start=(i == 0), stop=False)
                            nc.tensor.matmul(outr, lhsT=lhsr,
                                             rhs=rll_t[:, q, :],
                                             start=False,
                                             stop=(i == len(cruns) - 1))
                    # comp (in place), bf16 cast, prod, score slice
                    nc.vector.tensor_tensor(
                        out=src_rows[:, c0:c1, :], in0=src_rows[:, c0:c1, :],
                        in1=rel_ps[:, 0:gw, :], op=mybir.AluOpType.mult)
                    nc.scalar.copy(comp_bf[:, c0:c1, :], src_rows[:, c0:c1, :])
                    nc.vector.tensor_tensor(
                        out=scratch[:, c0:c1, :], in0=src_rows[:, c0:c1, :],
                        in1=drows_ps[:, 0:gw, :], op=mybir.AluOpType.mult)
                    nc.vector.tensor_reduce(
                        out=score[:, c0:c1], in_=scratch[:, c0:c1, :],
                        axis=mybir.AxisListType.X, op=mybir.AluOpType.add)

                nc.scalar.activation(
                    out=es[:, 0:S], in_=score[:, 0:S],
                    func=mybir.ActivationFunctionType.Exp)

                # es broadcast; comp_es = comp_bf * es_full (2x)
                es_ap = es[:, 0:S]
                nc.scalar.copy(es_full[:, 0:S, :],
                               bc(es_ap, [es_ap.ap[0], es_ap.ap[1], [0, H]]))
                nc.vector.tensor_tensor(
                    out=comp_es[:, 0:S, :], in0=comp_bf[:, 0:S, :],
                    in1=es_full[:, 0:S, :], op=mybir.AluOpType.mult)

                # accT[h, node] += comp_es_c^T @ ohtT_c ; den = ohtT^T @ es
                acct_ps = psa.tile([P, P], f32, tag="accT")
                ps_m = psa.tile([P, H], f32, tag="misc")
                for c in range(S):
                    nc.tensor.matmul(
                        acct_ps[:], lhsT=comp_es[:, c, :], rhs=ohtT_t[:, c, :],
                        start=(c == 0), stop=(c == S - 1))
                for c in range(S):
                    nc.tensor.matmul(
                        ps_m[:, 0:1], lhsT=ohtT_t[:, c, :], rhs=es[:, c:c + 1],
                        start=(c == 0), stop=(c == S - 1))

                den_sb = smallp.tile([P, 1], f32, tag="den_sb")
                nc.vector.tensor_scalar_max(den_sb[:], ps_m[:, 0:1], 1e-30)
                rden = smallp.tile([P, 1], f32, tag="rden")
                nc.vector.reciprocal(rden[:], den_sb[:])

                acct_sb = smallp.tile([P, P], f32, tag="acct_sb")
                nc.scalar.copy(acct_sb[:], acct_ps[:])
                nc.tensor.matmul(ps_m[:], lhsT=acct_sb[:], rhs=w_t[:],
                                 start=True, stop=True)
                out_sb = smallp.tile([P, H], f32, tag="out_sb")
                nc.scalar.activation(
                    out=out_sb[:], in_=ps_m[:],
                    func=mybir.ActivationFunctionType.Tanh, scale=rden[:])
                nc.sync.dma_start(out[base:base + nodes_b, :],
                                  out_sb[:nodes_b, :])
                coff += S
                roff0 += r_cols_b

    nc.compile()
    return nc


def _prep_inputs(ent_emb, rel_emb, neigh_w, src, dst, rel_id):
    import ml_dtypes
    src = np.asarray(src).astype(np.int64)
    dst = np.asarray(dst).astype(np.int64)
    rel_id = np.asarray(rel_id).astype(np.int64)
    n_blocks = (NPC + P - 1) // P

    order = np.argsort(dst, kind="stable")
    src_s, dst_s, rel_s = src[order], dst[order], rel_id[order]
    g_s = (dst_s // NPC) * n_blocks + (dst_s % NPC) // P
    n_gblocks = N_CORES * n_blocks
    bounds = np.searchsorted(g_s, np.arange(n_gblocks + 1))

    # per (core, block, section, q) edge lists, src-sorted within cell
    per = {}
    for c in range(N_CORES):
        for b in range(n_blocks):
            e0, e1 = bounds[c * n_blocks + b], bounds[c * n_blocks + b + 1]
            s_g, d_g, r_g = src_s[e0:e1], dst_s[e0:e1], rel_s[e0:e1]
            sec = (s_g >= LO_ROWS).astype(np.int64)
            q_g = r_g // P
            for s in (0, 1):
                for q in range(NQ):
                    m = (sec == s) & (q_g == q)
                    so = np.argsort(s_g[m], kind="stable")
                    per[(c, b, s, q)] = (s_g[m][so], d_g[m][so], r_g[m][so])

    cnt = {}
    for b in range(n_blocks):
        for s in (0, 1):
            for q in range(NQ):
                m = max(len(per[(c, b, s, q)][0]) for c in range(N_CORES))
                if s == 0 and q == 0:
                    m = max(m, 1)
                cnt[(b, s, q)] = m

    # slot layout per block: lo cells pad to 128-mult; hi cells likewise
    s_los, s_his, runs_all, layouts = [], [], [], []
    roff = 0
    for b in range(n_blocks):
        lo_n = sum(cnt[(b, 0, q)] for q in range(NQ))
        hi_n = sum(cnt[(b, 1, q)] for q in range(NQ))
        s_lo = max((lo_n + P - 1) // P, 1)
        s_hi = (hi_n + P - 1) // P
        s_los.append(s_lo)
        s_his.append(s_hi)
        lay = []
        for s, sbase, stot in ((0, 0, s_lo), (1, s_lo, s_hi)):
            pos = sbase * P
            for q in range(NQ):
                n = cnt[(b, s, q)]
                if n:
                    lay.append((q, pos, pos + n, s, False))
                    pos += n
            end = (sbase + stot) * P
            if pos < end and lay:
                lq, ls0, _ls1, lsec, lpad = lay[-1]
                if lsec == s and not lpad:
                    lay[-1] = (lq, ls0, end, s, False)
                else:
                    lay.append((lq, pos, end, s, True))
        layouts.append(lay)
        runs = []
        ro_b = 0  # block-local col offset into this block's roht slice
        for (q, s0, s1, _sec, _pad) in lay:
            c0, c1 = s0 // P, (s1 - 1) // P
            for c in range(c0, c1 + 1):
                a = max(s0, c * P)
                z = min(s1, (c + 1) * P)
                if a < z:
                    runs.append((c, q, a, z, ro_b))
                    ro_b += P
                    roff += P
        runs_all.append(runs)
    s_tot = [a + b for a, b in zip(s_los, s_his)]
    W_chunks = sum(s_tot)
    W = W_chunks * P
    R_cols = roff

    rel_pad = np.zeros((NQ * P, H), np.float32)
    rel_pad[:N_REL] = np.asarray(rel_emb, np.float32)
    rhi, rlo = _bfsplit(rel_pad)
    rlh = np.ascontiguousarray(rhi.reshape(NQ, P, H).transpose(1, 0, 2))
    rll = np.ascontiguousarray(rlo.reshape(NQ, P, H).transpose(1, 0, 2))

    ent_f32 = np.ascontiguousarray(ent_emb, np.float32)
    rng = np.arange(P)

    in_maps = []
    for cidx in range(N_CORES):
        sgi = np.zeros((W,), np.int16)
        dloc = np.full((W,), 255, np.int64)   # 255 = pad (no one-hot match)
        rloc = np.zeros((W,), np.int64)       # rel offset within q per slot
        coff = 0
        for b in range(n_blocks):
            o0 = coff * P
            for (q, s0, s1, sec, is_pad) in layouts[b]:
                ss, dd, rr = per[(cidx, b, sec, q)]
                if is_pad:
                    ss = ss[:0]; dd = dd[:0]; rr = rr[:0]
                n = len(ss)
                cap = s1 - s0
                assert n <= cap
                sub = LO_ROWS if sec == 1 else 0
                base = cidx * NPC + b * P
                if n:
                    sgi[o0 + s0:o0 + s0 + n] = (ss - sub).astype(np.int16)
                    dloc[o0 + s0:o0 + s0 + n] = dd - base
                    rloc[o0 + s0:o0 + s0 + n] = rr - q * P
                if n < cap:
                    sgi[o0 + s0 + n:o0 + s1] = np.int16((ss[0] - sub) if n else 0)
                    rloc[o0 + s0 + n:o0 + s1] = (rr[0] - q * P) if n else 0
            coff += s_tot[b]

        # gather idx layout per block (lo array || hi array)
        sgi_cols = []
        coff = 0
        for b in range(n_blocks):
            s_lo, s_hi, S = s_los[b], s_his[b], s_tot[b]
            o0 = coff * P
            lo_a = _idx_to_gather_layout(sgi[o0:o0 + s_lo * P])
            hi_a = (_idx_to_gather_layout(sgi[o0 + s_lo * P:o0 + S * P])
                    if s_hi > 0 else np.zeros((P, 0), np.int16))
            sgi_cols.append(np.concatenate([lo_a, hi_a], axis=1))
            coff += S
        sgi_l = np.concatenate(sgi_cols, axis=1)

        # one-hots (fp8: 0/1 exact)
        f8 = ml_dtypes.float8_e4m3
        oht = (dloc[None, :] == rng[:, None]).astype(f8)
        ohtT = np.zeros((P, W_chunks, P), f8)
        dl2 = dloc.reshape(W_chunks, P)  # [chunk, slot]
        valid = dl2 < P
        ci, si = np.nonzero(valid)
        ohtT[si, ci, dl2[ci, si]] = 1.0
        roht = np.zeros((P, R_cols), f8)
        boff = 0
        for b in range(n_blocks):
            blk0 = sum(s_tot[:b]) * P
            for (c, q, a, z, ro) in runs_all[b]:
                sl = np.arange(a, z)
                rl = rloc[blk0 + sl]
                roht[rl, boff + ro + (sl - c * P)] = 1.0
            boff += len(runs_all[b]) * P

        el = ent_f32[cidx * NPC:(cidx + 1) * NPC]
        ehi, elo = _bfsplit(el)

        in_maps.append({
            "ent": ent_f32,
            "ent_loc_hi": np.ascontiguousarray(ehi),
            "ent_loc_lo": np.ascontiguousarray(elo),
            "rel_hi": rlh,
            "rel_lo": rll,
            "w": np.ascontiguousarray(neigh_w, np.float32),
            "src_gi": np.ascontiguousarray(sgi_l),
            "oht": np.ascontiguousarray(oht),
            "ohtT": np.ascontiguousarray(ohtT),
            "roht": np.ascontiguousarray(roht),
        })
    key = (NPC, N_ENT, LO_ROWS, tuple(s_los), tuple(s_his),
           tuple(tuple(r) for r in sum(runs_all, [])))
    return in_maps, key, s_los, s_his, runs_all


LAST_RESULT = None


def _install_ntff_hook():
    import sys
    import types
    if "antenv.axon_hooks" in sys.modules:
        return
    mod = types.ModuleType("antenv.axon_hooks")
    hook = [None]
    mod.set_axon_ntff_profile_hook = lambda h: hook.__setitem__(0, h)
    mod.get_axon_ntff_profile_hook = lambda: hook[0]
    sys.modules["antenv.axon_hooks"] = mod
    import antenv
    antenv.axon_hooks = mod
    try:
        from trn_agent_boot.trn_boot import _ntff_profile_via_ctypes
        h = _ntff_profile_via_ctypes("/opt/axon/libaxon_pjrt.so")
        if h is not None:
            mod.set_axon_ntff_profile_hook(lambda *a, **k: h(*a, **k))
    except Exception as e:
        print("ntff hook install failed:", e)


def kernel(ent_emb, rel_emb, neigh_w, src, dst, rel_id, _trace=False):
    global LAST_RESULT
    from concourse.bass_utils import run_bass_kernel_spmd
    if _trace:
        _install_ntff_hook()

    in_maps, key, s_los, s_his, runs_all = _prep_inputs(
        ent_emb, rel_emb, neigh_w, src, dst, rel_id)
    if key not in _cache:
        _cache[key] = _build_program(NPC, N_ENT, LO_ROWS,
                                     s_los, s_his, runs_all)
    nc = _cache[key]
    res = run_bass_kernel_spmd(nc, in_maps, list(range(N_CORES)),
                               trace=_trace)
    LAST_RESULT = res
    return np.concatenate([r["out"] for r in res.results], axis=0)
